# revision 11
# baseline (speedup 1.0000x reference)
"""Trainium2 Bass kernel for: relu(1 - beta + x @ W^T).

Shapes (hardcoded): x [4096, 4096] f32, weights [4096, 4096] f32, beta [1] f32.
Output: [4096, 4096] f32.

Strategy: 8 cores arranged as a 4 (batch) x 2 (output) grid. Each core computes
out[bi*1024:(bi+1)*1024, oj*2048:(oj+1)*2048] = relu(1 - beta + x_shard @ W_shard^T).
Host pre-transposes x and W so the contraction dim (IN) lands on SBUF partitions
with fully contiguous DMA. Matmuls run as float32r (TF32-like full-rate fp32,
~1e-4 rel err) accumulating fp32 in PSUM; the ReLU + (1-beta) bias epilogue is
fused into one ScalarE activation reading PSUM directly.
"""
import numpy as np

import concourse.bass as bass
import concourse.mybir as mybir
import concourse.tile as tile
from concourse import bacc
from concourse.bass_utils import run_bass_kernel_spmd

BATCH = IN = OUT = 4096
GRID_B, GRID_O = 4, 2          # 8 cores
MB = BATCH // GRID_B           # 1024 batch rows per core
NO = OUT // GRID_O             # 2048 output cols per core
KT = IN // 128                 # 32 contraction tiles
NT = NO // 512                 # 4 output-col tiles per core
MT = MB // 128                 # 8 batch-row tiles per core

F32 = mybir.dt.float32
F32R = mybir.dt.float32r


def _build():
    nc = bacc.Bacc("TRN2", target_bir_lowering=False, debug=False)
    xT = nc.dram_tensor("xT", [IN, MB], F32, kind="ExternalInput").ap()
    wT = nc.dram_tensor("wT", [IN, NO], F32, kind="ExternalInput").ap()
    beta = nc.dram_tensor("beta", [128, 1], F32, kind="ExternalInput").ap()
    out = nc.dram_tensor("out", [MB, NO], F32, kind="ExternalOutput").ap()

    with tile.TileContext(nc) as tc:
        with (
            tc.tile_pool(name="xp", bufs=1) as xpool,
            tc.tile_pool(name="wp", bufs=12) as wpool,
            tc.tile_pool(name="op", bufs=2) as opool,
            tc.tile_pool(name="bp", bufs=1) as bpool,
            tc.tile_pool(name="pp", bufs=1, space="PSUM") as ppool,
        ):
            beta_t = bpool.tile([128, 1], F32, tag="beta")
            nc.sync.dma_start(beta_t[:], beta[:])
            bias_t = bpool.tile([128, 1], F32, tag="bias")
            # bias = (beta * -1) - (-1) = 1 - beta
            nc.vector.tensor_scalar(
                bias_t[:], beta_t[:], -1.0, -1.0,
                mybir.AluOpType.mult, mybir.AluOpType.subtract,
            )

            x_tiles = [None] * KT
            for j in range(NT):
                ps = [
                    ppool.tile([128, 512], F32, tag=f"ps{m}", name=f"ps{m}")
                    for m in range(MT)
                ]
                for kt in range(KT):
                    if j == 0:
                        xt = xpool.tile([128, MB], F32R, tag=f"x{kt}", name=f"x{kt}")
                        # First two k-tiles are split 4-way across SW DMA
                        # queues to cut first-matmul latency (single-queue
                        # latency for 512 KB is ~10 us); the rest go as one
                        # DMA each, round-robined across queues by Tile.
                        nchunks = 4 if kt < 2 else 1
                        cw = MB // nchunks
                        xeng = nc.gpsimd if kt % 2 == 0 else nc.scalar
                        for ci in range(nchunks):
                            xeng.dma_start(
                                xt[:, ci * cw:(ci + 1) * cw],
                                xT[kt * 128:(kt + 1) * 128,
                                   ci * cw:(ci + 1) * cw].bitcast(F32R),
                            )
                        x_tiles[kt] = xt
                    wt = wpool.tile([128, 512], F32R, tag="w", name="wt")
                    wchunks = 2 if (j == 0 and kt < 2) else 1
                    wcw = 512 // wchunks
                    for ci in range(wchunks):
                        nc.sync.dma_start(
                            wt[:, ci * wcw:(ci + 1) * wcw],
                            wT[kt * 128:(kt + 1) * 128,
                               j * 512 + ci * wcw:j * 512 + (ci + 1) * wcw
                               ].bitcast(F32R),
                        )
                    for m in range(MT):
                        nc.tensor.matmul(
                            ps[m][:],
                            x_tiles[kt][:, m * 128:(m + 1) * 128],
                            wt[:],
                            start=(kt == 0),
                            stop=(kt == KT - 1),
                        )
                ot = opool.tile([128, MT, 512], F32, tag="o", name="ot")
                for m in range(MT):
                    # Alternate ReLU+bias between ScalarE and VectorE so the
                    # epilogue doesn't serialize on one engine.
                    if m % 2 == 0:
                        nc.scalar.activation(
                            ot[:, m, :], ps[m][:], mybir.ActivationFunctionType.Relu,
                            bias=bias_t[:], scale=1.0,
                        )
                    else:
                        nc.vector.tensor_scalar(
                            ot[:, m, :], ps[m][:], bias_t[:], 0.0,
                            mybir.AluOpType.add, mybir.AluOpType.max,
                        )
                if j < NT - 1:
                    half = MT // 2
                    for h in range(2):
                        nc.gpsimd.dma_start(
                            out[h * half * 128:(h + 1) * half * 128,
                                j * 512:(j + 1) * 512].rearrange(
                                "(m p) c -> p m c", p=128
                            ),
                            ot[:, h * half:(h + 1) * half, :],
                        )
                else:
                    # Last pass: store per m-tile so the final transfer after
                    # the last ReLU is small; queues round-robin in parallel.
                    for m in range(MT):
                        nc.gpsimd.dma_start(
                            out[m * 128:(m + 1) * 128, j * 512:(j + 1) * 512],
                            ot[:, m, :],
                        )
    nc.compile()
    return nc


_NC_CACHE = None


def _get_nc():
    global _NC_CACHE
    if _NC_CACHE is None:
        _NC_CACHE = _build()
    return _NC_CACHE


def kernel(x, weights, beta, _trace=False, _results_out=None):
    x = np.asarray(x, dtype=np.float32)
    weights = np.asarray(weights, dtype=np.float32)
    beta = np.asarray(beta, dtype=np.float32)

    xT = np.ascontiguousarray(x.T)        # [IN, BATCH]
    wT = np.ascontiguousarray(weights.T)  # [IN, OUT]
    beta_b = np.ascontiguousarray(
        np.broadcast_to(beta.reshape(1, 1), (128, 1)).astype(np.float32)
    )

    in_maps = []
    for c in range(GRID_B * GRID_O):
        bi, oj = divmod(c, GRID_O)
        in_maps.append({
            "xT": np.ascontiguousarray(xT[:, bi * MB:(bi + 1) * MB]),
            "wT": np.ascontiguousarray(wT[:, oj * NO:(oj + 1) * NO]),
            "beta": beta_b,
        })

    nc = _get_nc()
    res = run_bass_kernel_spmd(
        nc, in_maps, core_ids=list(range(8)), trace=_trace,
        trace_cores=list(range(8)) if _trace else None,
    )
    if _results_out is not None:
        _results_out.append(res)

    out = np.empty((BATCH, OUT), dtype=np.float32)
    for c in range(GRID_B * GRID_O):
        bi, oj = divmod(c, GRID_O)
        out[bi * MB:(bi + 1) * MB, oj * NO:(oj + 1) * NO] = res.results[c]["out"]
    return out


# revision 13
# speedup vs baseline: 1.0239x; 1.0239x over previous
"""Trainium2 Bass kernel for: relu(1 - beta + x @ W^T).

Shapes (hardcoded): x [4096, 4096] f32, weights [4096, 4096] f32, beta [1] f32.
Output: [4096, 4096] f32.

Strategy: 8 cores arranged as a 4 (batch) x 2 (output) grid. Each core computes
out[bi*1024:(bi+1)*1024, oj*2048:(oj+1)*2048] = relu(1 - beta + x_shard @ W_shard^T).
Host pre-transposes x and W so the contraction dim (IN) lands on SBUF partitions
with fully contiguous DMA. Matmuls run as float32r (TF32-like full-rate fp32,
~1e-4 rel err) accumulating fp32 in PSUM; the ReLU + (1-beta) bias epilogue is
fused into one ScalarE activation reading PSUM directly.
"""
import numpy as np

import concourse.bass as bass
import concourse.mybir as mybir
import concourse.tile as tile
from concourse import bacc
from concourse.bass_utils import run_bass_kernel_spmd

BATCH = IN = OUT = 4096
GRID_B, GRID_O = 4, 2          # 8 cores
MB = BATCH // GRID_B           # 1024 batch rows per core
NO = OUT // GRID_O             # 2048 output cols per core
KT = IN // 128                 # 32 contraction tiles
NT = NO // 512                 # 4 output-col tiles per core
MT = MB // 128                 # 8 batch-row tiles per core

F32 = mybir.dt.float32
F32R = mybir.dt.float32r


def _build():
    nc = bacc.Bacc("TRN2", target_bir_lowering=False, debug=False)
    xT = nc.dram_tensor("xT", [IN, MB], F32, kind="ExternalInput").ap()
    wT = nc.dram_tensor("wT", [IN, NO], F32, kind="ExternalInput").ap()
    beta = nc.dram_tensor("beta", [128, 1], F32, kind="ExternalInput").ap()
    out = nc.dram_tensor("out", [MB, NO], F32, kind="ExternalOutput").ap()

    with tile.TileContext(nc) as tc:
        with (
            tc.tile_pool(name="xp", bufs=1) as xpool,
            tc.tile_pool(name="wp", bufs=12) as wpool,
            tc.tile_pool(name="op", bufs=2) as opool,
            tc.tile_pool(name="bp", bufs=1) as bpool,
            tc.tile_pool(name="pp", bufs=1, space="PSUM") as ppool,
        ):
            beta_t = bpool.tile([128, 1], F32, tag="beta")
            nc.sync.dma_start(beta_t[:], beta[:])
            bias_t = bpool.tile([128, 1], F32, tag="bias")
            # bias = (beta * -1) - (-1) = 1 - beta
            nc.vector.tensor_scalar(
                bias_t[:], beta_t[:], -1.0, -1.0,
                mybir.AluOpType.mult, mybir.AluOpType.subtract,
            )

            x_tiles = [None] * KT
            for j in range(NT):
                ps = [
                    ppool.tile([128, 512], F32, tag=f"ps{m}", name=f"ps{m}")
                    for m in range(MT)
                ]
                for kt in range(KT):
                    if j == 0:
                        xt = xpool.tile([128, MB], F32R, tag=f"x{kt}", name=f"x{kt}")
                        # First two k-tiles are split 4-way across SW DMA
                        # queues to cut first-matmul latency (single-queue
                        # latency for 512 KB is ~10 us); the rest go as one
                        # DMA each, round-robined across queues by Tile.
                        nchunks = 4 if kt < 2 else 1
                        cw = MB // nchunks
                        for ci in range(nchunks):
                            # kt<2 startup chunks split across gpsimd+scalar
                            # issue queues (both idle at t=0) for low latency.
                            eng = nc.scalar if (kt < 2 and ci % 2 == 1) else nc.gpsimd
                            eng.dma_start(
                                xt[:, ci * cw:(ci + 1) * cw],
                                xT[kt * 128:(kt + 1) * 128,
                                   ci * cw:(ci + 1) * cw].bitcast(F32R),
                            )
                        x_tiles[kt] = xt
                    wt = wpool.tile([128, 512], F32R, tag="w", name="wt")
                    wchunks = 2 if (j == 0 and kt < 2) else 1
                    wcw = 512 // wchunks
                    for ci in range(wchunks):
                        nc.sync.dma_start(
                            wt[:, ci * wcw:(ci + 1) * wcw],
                            wT[kt * 128:(kt + 1) * 128,
                               j * 512 + ci * wcw:j * 512 + (ci + 1) * wcw
                               ].bitcast(F32R),
                        )
                    for m in range(MT):
                        nc.tensor.matmul(
                            ps[m][:],
                            x_tiles[kt][:, m * 128:(m + 1) * 128],
                            wt[:],
                            start=(kt == 0),
                            stop=(kt == KT - 1),
                        )
                ot = opool.tile([128, MT, 512], F32, tag="o", name="ot")
                for m in range(MT):
                    # Alternate ReLU+bias between ScalarE and VectorE so the
                    # epilogue doesn't serialize on one engine.
                    if m % 2 == 0:
                        nc.scalar.activation(
                            ot[:, m, :], ps[m][:], mybir.ActivationFunctionType.Relu,
                            bias=bias_t[:], scale=1.0,
                        )
                    else:
                        nc.vector.tensor_scalar(
                            ot[:, m, :], ps[m][:], bias_t[:], 0.0,
                            mybir.AluOpType.add, mybir.AluOpType.max,
                        )
                if j < NT - 1:
                    half = MT // 2
                    for h in range(2):
                        nc.gpsimd.dma_start(
                            out[h * half * 128:(h + 1) * half * 128,
                                j * 512:(j + 1) * 512].rearrange(
                                "(m p) c -> p m c", p=128
                            ),
                            ot[:, h * half:(h + 1) * half, :],
                        )
                else:
                    # Last pass: store per m-tile so the final transfer after
                    # the last ReLU is small; queues round-robin in parallel.
                    for m in range(MT):
                        nc.gpsimd.dma_start(
                            out[m * 128:(m + 1) * 128, j * 512:(j + 1) * 512],
                            ot[:, m, :],
                        )
    nc.compile()
    return nc


_NC_CACHE = None


def _get_nc():
    global _NC_CACHE
    if _NC_CACHE is None:
        _NC_CACHE = _build()
    return _NC_CACHE


def kernel(x, weights, beta, _trace=False, _results_out=None):
    x = np.asarray(x, dtype=np.float32)
    weights = np.asarray(weights, dtype=np.float32)
    beta = np.asarray(beta, dtype=np.float32)

    xT = np.ascontiguousarray(x.T)        # [IN, BATCH]
    wT = np.ascontiguousarray(weights.T)  # [IN, OUT]
    beta_b = np.ascontiguousarray(
        np.broadcast_to(beta.reshape(1, 1), (128, 1)).astype(np.float32)
    )

    in_maps = []
    for c in range(GRID_B * GRID_O):
        bi, oj = divmod(c, GRID_O)
        in_maps.append({
            "xT": np.ascontiguousarray(xT[:, bi * MB:(bi + 1) * MB]),
            "wT": np.ascontiguousarray(wT[:, oj * NO:(oj + 1) * NO]),
            "beta": beta_b,
        })

    nc = _get_nc()
    res = run_bass_kernel_spmd(
        nc, in_maps, core_ids=list(range(8)), trace=_trace,
        trace_cores=list(range(8)) if _trace else None,
    )
    if _results_out is not None:
        _results_out.append(res)

    out = np.empty((BATCH, OUT), dtype=np.float32)
    for c in range(GRID_B * GRID_O):
        bi, oj = divmod(c, GRID_O)
        out[bi * MB:(bi + 1) * MB, oj * NO:(oj + 1) * NO] = res.results[c]["out"]
    return out


# revision 14
# speedup vs baseline: 1.0627x; 1.0379x over previous
"""Trainium2 Bass kernel for: relu(1 - beta + x @ W^T).

Shapes (hardcoded): x [4096, 4096] f32, weights [4096, 4096] f32, beta [1] f32.
Output: [4096, 4096] f32.

Strategy: 8 cores arranged as a 4 (batch) x 2 (output) grid. Each core computes
out[bi*1024:(bi+1)*1024, oj*2048:(oj+1)*2048] = relu(1 - beta + x_shard @ W_shard^T).
Host pre-transposes x and W so the contraction dim (IN) lands on SBUF partitions
with fully contiguous DMA. Matmuls run as float32r (TF32-like full-rate fp32,
~1e-4 rel err) accumulating fp32 in PSUM; the ReLU + (1-beta) bias epilogue is
fused into one ScalarE activation reading PSUM directly.
"""
import numpy as np

import concourse.bass as bass
import concourse.mybir as mybir
import concourse.tile as tile
from concourse import bacc
from concourse.bass_utils import run_bass_kernel_spmd

BATCH = IN = OUT = 4096
GRID_B, GRID_O = 4, 2          # 8 cores
MB = BATCH // GRID_B           # 1024 batch rows per core
NO = OUT // GRID_O             # 2048 output cols per core
KT = IN // 128                 # 32 contraction tiles
NT = NO // 512                 # 4 output-col tiles per core
MT = MB // 128                 # 8 batch-row tiles per core

F32 = mybir.dt.float32
F32R = mybir.dt.float32r


def _build():
    nc = bacc.Bacc("TRN2", target_bir_lowering=False, debug=False)
    xT = nc.dram_tensor("xT", [IN, MB], F32, kind="ExternalInput").ap()
    wT = nc.dram_tensor("wT", [IN, NO], F32, kind="ExternalInput").ap()
    beta = nc.dram_tensor("beta", [128, 1], F32, kind="ExternalInput").ap()
    out = nc.dram_tensor("out", [MB, NO], F32, kind="ExternalOutput").ap()

    with tile.TileContext(nc) as tc:
        with (
            tc.tile_pool(name="xp", bufs=1) as xpool,
            tc.tile_pool(name="wp", bufs=12) as wpool,
            tc.tile_pool(name="op", bufs=2) as opool,
            tc.tile_pool(name="bp", bufs=1) as bpool,
            tc.tile_pool(name="pp", bufs=1, space="PSUM") as ppool,
        ):
            beta_t = bpool.tile([128, 1], F32, tag="beta")
            nc.sync.dma_start(beta_t[:], beta[:])
            bias_t = bpool.tile([128, 1], F32, tag="bias")
            # bias = (beta * -1) - (-1) = 1 - beta
            nc.vector.tensor_scalar(
                bias_t[:], beta_t[:], -1.0, -1.0,
                mybir.AluOpType.mult, mybir.AluOpType.subtract,
            )

            x_tiles = [None] * KT
            for j in range(NT):
                ps = [
                    ppool.tile([128, 512], F32, tag=f"ps{m}", name=f"ps{m}")
                    for m in range(MT)
                ]
                for kt in range(KT):
                    if j == 0:
                        xt = xpool.tile([128, MB], F32R, tag=f"x{kt}", name=f"x{kt}")
                        # First two k-tiles are split 4-way across SW DMA
                        # queues to cut first-matmul latency (single-queue
                        # latency for 512 KB is ~10 us); the rest go as one
                        # DMA each, round-robined across queues by Tile.
                        nchunks = 4 if kt < 2 else 1
                        cw = MB // nchunks
                        for ci in range(nchunks):
                            # kt<2 startup chunks split across gpsimd+scalar
                            # issue queues (both idle at t=0) for low latency.
                            eng = nc.scalar if (kt < 2 and ci % 2 == 1) else nc.gpsimd
                            eng.dma_start(
                                xt[:, ci * cw:(ci + 1) * cw],
                                xT[kt * 128:(kt + 1) * 128,
                                   ci * cw:(ci + 1) * cw].bitcast(F32R),
                            )
                        x_tiles[kt] = xt
                    wt = wpool.tile([128, 512], F32R, tag="w", name="wt")
                    wchunks = 2 if (j == 0 and kt < 2) else 1
                    wcw = 512 // wchunks
                    for ci in range(wchunks):
                        nc.sync.dma_start(
                            wt[:, ci * wcw:(ci + 1) * wcw],
                            wT[kt * 128:(kt + 1) * 128,
                               j * 512 + ci * wcw:j * 512 + (ci + 1) * wcw
                               ].bitcast(F32R),
                        )
                    for m in range(MT):
                        nc.tensor.matmul(
                            ps[m][:],
                            x_tiles[kt][:, m * 128:(m + 1) * 128],
                            wt[:],
                            start=(kt == 0),
                            stop=(kt == KT - 1),
                        )
                ot = opool.tile([128, MT, 512], F32, tag="o", name="ot")
                for m in range(MT):
                    # Alternate ReLU+bias between ScalarE and VectorE so the
                    # epilogue doesn't serialize on one engine.
                    if m % 2 == 0:
                        nc.scalar.activation(
                            ot[:, m, :], ps[m][:], mybir.ActivationFunctionType.Relu,
                            bias=bias_t[:], scale=1.0,
                        )
                    else:
                        nc.vector.tensor_scalar(
                            ot[:, m, :], ps[m][:], bias_t[:], 0.0,
                            mybir.AluOpType.add, mybir.AluOpType.max,
                        )
                if j < NT - 1:
                    half = MT // 2
                    for h in range(2):
                        nc.gpsimd.dma_start(
                            out[h * half * 128:(h + 1) * half * 128,
                                j * 512:(j + 1) * 512].rearrange(
                                "(m p) c -> p m c", p=128
                            ),
                            ot[:, h * half:(h + 1) * half, :],
                        )
                else:
                    # Last pass: the store latency lands on the kernel tail, so
                    # spread it over both SW (gpsimd) and HW (sync/scalar) DMA
                    # queues, with the final m-tiles split finest so the last
                    # transfer after the last ReLU is as small as possible.
                    store_engs = [nc.gpsimd, nc.sync]
                    ei = 0
                    for m in range(MT):
                        nch = 1 if m < MT - 2 else (2 if m == MT - 2 else 4)
                        cw = 512 // nch
                        for ci in range(nch):
                            eng = store_engs[ei % 2] if m < MT - 1 else (
                                [nc.gpsimd, nc.sync, nc.scalar, nc.gpsimd][ci]
                            )
                            ei += 1
                            eng.dma_start(
                                out[m * 128:(m + 1) * 128,
                                    j * 512 + ci * cw:j * 512 + (ci + 1) * cw],
                                ot[:, m, ci * cw:(ci + 1) * cw],
                            )
    nc.compile()
    return nc


_NC_CACHE = None


def _get_nc():
    global _NC_CACHE
    if _NC_CACHE is None:
        _NC_CACHE = _build()
    return _NC_CACHE


def kernel(x, weights, beta, _trace=False, _results_out=None):
    x = np.asarray(x, dtype=np.float32)
    weights = np.asarray(weights, dtype=np.float32)
    beta = np.asarray(beta, dtype=np.float32)

    xT = np.ascontiguousarray(x.T)        # [IN, BATCH]
    wT = np.ascontiguousarray(weights.T)  # [IN, OUT]
    beta_b = np.ascontiguousarray(
        np.broadcast_to(beta.reshape(1, 1), (128, 1)).astype(np.float32)
    )

    in_maps = []
    for c in range(GRID_B * GRID_O):
        bi, oj = divmod(c, GRID_O)
        in_maps.append({
            "xT": np.ascontiguousarray(xT[:, bi * MB:(bi + 1) * MB]),
            "wT": np.ascontiguousarray(wT[:, oj * NO:(oj + 1) * NO]),
            "beta": beta_b,
        })

    nc = _get_nc()
    res = run_bass_kernel_spmd(
        nc, in_maps, core_ids=list(range(8)), trace=_trace,
        trace_cores=list(range(8)) if _trace else None,
    )
    if _results_out is not None:
        _results_out.append(res)

    out = np.empty((BATCH, OUT), dtype=np.float32)
    for c in range(GRID_B * GRID_O):
        bi, oj = divmod(c, GRID_O)
        out[bi * MB:(bi + 1) * MB, oj * NO:(oj + 1) * NO] = res.results[c]["out"]
    return out
